# revision 16
# baseline (speedup 1.0000x reference)
"""GPT-2 (6L, D=768, H=12, B=2, T=1024, V=50257) forward pass on 8 trn2 cores.

Sharding: tokens 2048 -> 256/core (cores 0-3 = batch 0, 4-7 = batch 1).
Attention: per-layer AllGather of K/V (bf16) within each 4-core batch group;
every core computes full-kv attention for its 256 queries with an additive
mask (SPMD-uniform work). Logits: vocab-sharded (6656 padded cols/core)
against an AllGathered final hidden state; host concatenates shards.
Matmuls run in bf16 (fp32 PSUM accumulation); the residual stream, LN
statistics and pre-softmax scores stay fp32. During each collective the PE
runs filler matmuls so the HAM clock gate never re-throttles to 1.2 GHz.
"""

import sys
from contextlib import ExitStack

import numpy as np
import ml_dtypes

sys.path.insert(0, "/opt/trn_rl_repo")

import concourse.bass as bass
import concourse.tile as tile
from concourse import bacc, mybir
from concourse.bass_utils import run_bass_kernel_spmd

F32 = mybir.dt.float32
F32R = mybir.dt.float32r
BF = mybir.dt.bfloat16
AF = mybir.ActivationFunctionType
ALU = mybir.AluOpType

L, D, V, B, T, H, HD = 6, 768, 50257, 2, 1024, 12, 64
NTOK = 256           # tokens per core
NC = 8               # cores
KT = D // 128        # 6 feature tiles
VSHARD = 6656        # padded vocab per core (13 * 512); 8*6656 = 53248
VT = VSHARD // 512   # 13
TT = (B * T) // 128  # 16 token tiles of the full sequence
MASKVAL = -240.0     # pre-scale additive mask (-30 after 1/8 scaling)
FILL_LAYER = 300     # warm-filler matmuls issued during each KV AllGather
FILL_FINAL = 330     # and during the final hidden-state AllGather

_CACHE = {}


def _r(x):
    return x


def build_nc(debug=False):
    nc = bacc.Bacc("TRN2", target_bir_lowering=False, debug=False, num_devices=NC)

    # ---- per-core inputs ----
    x0T = nc.dram_tensor("x0T", [D, NTOK], F32R, kind="ExternalInput")
    onesd = nc.dram_tensor("onesd", [128, 65], F32R, kind="ExternalInput")
    onesb = nc.dram_tensor("onesb", [128, 512], BF, kind="ExternalInput")
    mask8 = nc.dram_tensor("mask8", [T, NTOK], F32, kind="ExternalInput")
    wteT = nc.dram_tensor("wteT", [KT, VT, 128, 512], BF, kind="ExternalInput")
    # ---- replicated weights ----
    wqk = nc.dram_tensor("wqk", [L, 12, 128, KT, 128], BF, kind="ExternalInput")
    wv = nc.dram_tensor("wv", [L, 2, 128, KT, 384], BF, kind="ExternalInput")
    wproj = nc.dram_tensor("wproj", [L, KT, 128, KT, 128], BF, kind="ExternalInput")
    wfc = nc.dram_tensor("wfc", [L, 24, 128, KT, 128], BF, kind="ExternalInput")
    wfc2 = nc.dram_tensor("wfc2", [L, KT, 128, 24, 128], BF, kind="ExternalInput")
    b_qkv = nc.dram_tensor("b_qkv", [L, 128, 12], F32, kind="ExternalInput")
    b_v = nc.dram_tensor("b_v", [L, 768], F32, kind="ExternalInput")
    b_proj = nc.dram_tensor("b_proj", [L, 128, KT], F32, kind="ExternalInput")
    b_fc = nc.dram_tensor("b_fc", [L, 128, 24], F32, kind="ExternalInput")
    b_fc2 = nc.dram_tensor("b_fc2", [L, 128, KT], F32, kind="ExternalInput")
    s_ln1 = nc.dram_tensor("s_ln1", [L, 128, KT], F32, kind="ExternalInput")
    bi_ln1 = nc.dram_tensor("bi_ln1", [L, 128, KT], F32, kind="ExternalInput")
    s_ln2 = nc.dram_tensor("s_ln2", [L, 128, KT], F32, kind="ExternalInput")
    bi_ln2 = nc.dram_tensor("bi_ln2", [L, 128, KT], F32, kind="ExternalInput")
    s_lnf = nc.dram_tensor("s_lnf", [128, KT], F32, kind="ExternalInput")
    bi_lnf = nc.dram_tensor("bi_lnf", [128, KT], F32, kind="ExternalInput")
    # ---- outputs ----
    out = nc.dram_tensor("out", [B * T, VSHARD], BF, kind="ExternalOutput")
    warm_sink = nc.dram_tensor("warm_sink", [1, 4], F32, kind="ExternalOutput")

    # ---- collective bounce buffers (bf16 payloads) ----
    KVSZ = D * NTOK  # 196608 elems for k (and again for v)
    kv_in = nc.dram_tensor("kv_in", [2 * KVSZ], BF)
    kv_out = nc.dram_tensor("kv_out", [8 * KVSZ], BF)
    xf_in = nc.dram_tensor("xf_in", [KVSZ], BF)
    xf_out = nc.dram_tensor("xf_out", [NC * KVSZ], BF, addr_space="Shared")
    kv_groups = [[0, 1, 2, 3], [4, 5, 6, 7]]

    with tile.TileContext(nc) as tc, ExitStack() as ctx:
        const = ctx.enter_context(tc.tile_pool(name="const", bufs=1))
        ones = const.tile([128, 1], F32R)
        nc.sync.dma_start(out=ones, in_=onesd[:, 0:1])
        eps = const.tile([1, 1], F32)
        nc.vector.memset(eps, 1e-5)
        warm_a = const.tile([128, 128], BF)
        nc.sync.dma_start(out=warm_a, in_=onesb.ap()[:, 0:128])
        warm_b = const.tile([128, 512], BF)
        nc.sync.dma_start(out=warm_b, in_=onesb.ap())
        mask_sb = const.tile([128, 8, NTOK], F32)
        nc.sync.dma_start(
            out=mask_sb,
            in_=mask8.ap().rearrange("(n p) t -> p n t", p=128),
        )
        scl = const.tile([128, 4 * L + 2, KT], F32)  # ln scales/biases
        for l in range(L):
            nc.sync.dma_start(out=scl[:, 4 * l + 0, :], in_=s_ln1[l])
            nc.sync.dma_start(out=scl[:, 4 * l + 1, :], in_=bi_ln1[l])
            nc.sync.dma_start(out=scl[:, 4 * l + 2, :], in_=s_ln2[l])
            nc.sync.dma_start(out=scl[:, 4 * l + 3, :], in_=bi_ln2[l])
        nc.sync.dma_start(out=scl[:, 4 * L + 0, :], in_=s_lnf.ap())
        nc.sync.dma_start(out=scl[:, 4 * L + 1, :], in_=bi_lnf.ap())
        bias_sb = const.tile([128, L, 12 + KT + 24 + KT], F32)
        for l in range(L):
            nc.sync.dma_start(out=bias_sb[:, l, 0:12], in_=b_qkv[l])
            nc.sync.dma_start(out=bias_sb[:, l, 12 : 12 + KT], in_=b_proj[l])
            nc.sync.dma_start(out=bias_sb[:, l, 18:42], in_=b_fc[l])
            nc.sync.dma_start(out=bias_sb[:, l, 42:48], in_=b_fc2[l])

        with ExitStack() as body:
            resid = body.enter_context(tc.tile_pool(name="resid", bufs=2))
            lnp = body.enter_context(tc.tile_pool(name="lnp", bufs=1))
            qkvp = body.enter_context(tc.tile_pool(name="qkvp", bufs=1))
            kvp = body.enter_context(tc.tile_pool(name="kvp", bufs=1))
            wpool = body.enter_context(tc.tile_pool(name="wpool", bufs=1))
            w2pool = body.enter_context(tc.tile_pool(name="w2pool", bufs=2))
            att = body.enter_context(tc.tile_pool(name="att", bufs=4))
            yp = body.enter_context(tc.tile_pool(name="yp", bufs=1))
            hp = body.enter_context(tc.tile_pool(name="hp", bufs=1))
            stat = body.enter_context(tc.tile_pool(name="stat", bufs=1))
            ps_mm = body.enter_context(tc.tile_pool(name="ps_mm", bufs=2, space="PSUM"))
            ps_s = body.enter_context(tc.tile_pool(name="ps_s", bufs=2, space="PSUM"))
            ps_av = body.enter_context(tc.tile_pool(name="ps_av", bufs=2, space="PSUM"))
            ps_st = body.enter_context(tc.tile_pool(name="ps_st", bufs=1, space="PSUM"))

            # warm-filler scratch: reuses the ps_st pool so PSUM stays at 8 banks
            ps_warm = ps_st.tile([128, 512], F32, name="warm", tag="warm")
            warm_out = stat.tile([1, 4], F32, name="warm_out", tag="warm_out")

            def warm_fill(n):
                for _ in range(n):
                    nc.tensor.matmul(ps_warm, _r(warm_a), _r(warm_b),
                                     start=True, stop=True)
                # liveness anchor: keep the fillers from being dead-code
                # eliminated (warm_out is DMA'd to warm_sink at the end)
                nc.scalar.copy(warm_out, ps_warm[0:1, 0:4])

            x_tiles = []
            for j in range(KT):
                xt = resid.tile([128, NTOK], F32R, name=f"x{j}", tag=f"x{j}")
                nc.sync.dma_start(out=xt, in_=x0T[j * 128 : (j + 1) * 128, :])
                x_tiles.append(xt)

            def layernorm(xs, s_col, b_col):
                ln_ps = ps_st.tile([1, 2, NTOK], F32, name="ln_ps", tag="ln_ps")
                sums = ln_ps[:, 0, :]
                sumq = ln_ps[:, 1, :]
                sq = []
                for j in range(KT):
                    sqt = lnp.tile([128, NTOK], F32R, name=f"sq{j}", tag=f"sq{j}")
                    nc.vector.tensor_mul(sqt, xs[j], xs[j])
                    sq.append(sqt)
                for j in range(KT):
                    nc.tensor.matmul(sums, _r(ones), _r(xs[j]),
                                     start=(j == 0), stop=(j == KT - 1))
                for j in range(KT):
                    nc.tensor.matmul(sumq, _r(ones), _r(sq[j]),
                                     start=(j == 0), stop=(j == KT - 1))
                mu_t = stat.tile([1, NTOK], F32, name="mu_t", tag="mu_t")
                rs_t = stat.tile([1, NTOK], F32, name="rs_t", tag="rs_t")
                mu = mu_t[0:1, :]
                rs = rs_t[0:1, :]
                nc.scalar.mul(mu, sums, 1.0 / D)
                musq = stat.tile([1, NTOK], F32, name="musq", tag="musq")
                nc.vector.tensor_mul(musq, mu, mu)
                var = stat.tile([1, NTOK], F32, name="var", tag="var")
                nc.vector.tensor_scalar(out=var, in0=sumq, scalar1=1.0 / D,
                                        scalar2=None, op0=ALU.mult)
                nc.vector.tensor_sub(var, var, musq)
                nc.scalar.activation(var, var, AF.Sqrt, bias=eps)
                nc.vector.reciprocal(rs, var)
                bc = stat.tile([128, 2, NTOK], F32, name="bc", tag="bc")
                nc.gpsimd.partition_broadcast(bc[:, 0, :], mu)
                nc.gpsimd.partition_broadcast(bc[:, 1, :], rs)
                outs = []
                for j in range(KT):
                    tmp = lnp.tile([128, NTOK], BF, name=f"lt{j}", tag=f"lt{j}")
                    nc.vector.tensor_sub(tmp, xs[j], bc[:, 0, :])
                    nc.vector.tensor_mul(tmp, tmp, bc[:, 1, :])
                    ot = lnp.tile([128, NTOK], BF, name=f"ln{j}", tag=f"ln{j}")
                    nc.vector.tensor_scalar(out=ot, in0=tmp,
                                            scalar1=s_col[:, j : j + 1],
                                            scalar2=b_col[:, j : j + 1],
                                            op0=ALU.mult, op1=ALU.add)
                    outs.append(ot)
                return outs

            kv_in_ap = kv_in.ap()
            k_in = kv_in_ap[0:KVSZ].rearrange("(p t) -> p t", p=D)
            v_in = kv_in_ap[KVSZ:].rearrange("(t d) -> t d", t=NTOK)

            for l in range(L):
                sc = scl[:, 4 * l + 0, :]
                bc1 = scl[:, 4 * l + 1, :]
                ln1 = layernorm(x_tiles, sc, bc1)

                # --- QK part of QKV: feature-major out [1536, 256] ---
                qkT = []
                for ot in range(12):
                    wt = wpool.tile([128, KT, 128], BF, name="wqk", tag="wqk", bufs=2)
                    nc.sync.dma_start(out=wt, in_=wqk[l, ot])
                    ps = ps_mm.tile([128, NTOK], F32, name="mm", tag="mm")
                    for j in range(KT):
                        nc.tensor.matmul(
                            ps, _r(wt[:, j, :]),
                            _r(ln1[j]), start=(j == 0), stop=(j == KT - 1))
                    sb = qkvp.tile([128, NTOK], BF, name=f"qk{ot}", tag=f"qk{ot}")
                    nc.vector.tensor_scalar_add(sb, ps, bias_sb[:, l, ot : ot + 1])
                    qkT.append(sb)
                    if ot >= 6:  # k tiles -> collective input
                        nc.sync.dma_start(
                            out=k_in[(ot - 6) * 128 : (ot - 5) * 128, :], in_=sb)

                # --- V part: token-major out [256, 768] ---
                bv_sb = wpool.tile([128, 768], F32, name="bv", tag="bv")
                bvl = b_v.ap()[l]
                nc.sync.dma_start(
                    out=bv_sb,
                    in_=bass.AP(tensor=bvl.tensor, offset=bvl.offset,
                                ap=[[0, 128]] + list(bvl.ap)),
                )
                vloc = [qkvp.tile([128, 768], BF, name=f"vloc{tt}", tag=f"vloc{tt}")
                        for tt in range(2)]
                for oh in range(2):
                    wt = wpool.tile([128, KT, 384], BF, name="wvt", tag="wvt", bufs=2)
                    nc.sync.dma_start(out=wt, in_=wv[l, oh])
                    for tt in range(2):
                        ps = ps_mm.tile([128, 384], F32, name="mmv", tag="mm")
                        for j in range(KT):
                            nc.tensor.matmul(
                                ps, _r(ln1[j][:, tt * 128 : (tt + 1) * 128]),
                                _r(wt[:, j, :]),
                                start=(j == 0), stop=(j == KT - 1))
                        nc.vector.tensor_add(
                            vloc[tt][:, oh * 384 : (oh + 1) * 384], ps,
                            bv_sb[:, oh * 384 : (oh + 1) * 384])
                for tt in range(2):
                    nc.sync.dma_start(
                        out=v_in[tt * 128 : (tt + 1) * 128, :], in_=vloc[tt])

                nc.gpsimd.collective_compute(
                    "AllGather", ALU.bypass, replica_groups=kv_groups,
                    ins=[kv_in.ap()], outs=[kv_out.ap()[0 : 4 * 2 * KVSZ]],
                )
                warm_fill(FILL_LAYER)

                # --- load gathered K (feature-major [768, 1024]) and V ---
                k_sb = [kvp.tile([128, T], BF, name=f"k{j}", tag=f"k{j}") for j in range(KT)]
                v_sb = [kvp.tile([128, 12, 65], BF, name=f"v{j}", tag=f"v{j}") for j in range(8)]
                kvo = kv_out.ap()
                for r in range(4):
                    base = r * 2 * KVSZ
                    k_r = kvo[base : base + KVSZ].rearrange("(p t) -> p t", p=D)
                    v_r = kvo[base + KVSZ : base + 2 * KVSZ].rearrange(
                        "(t h d) -> t h d", t=NTOK, h=12)
                    for j in range(KT):
                        nc.sync.dma_start(
                            out=k_sb[j][:, r * NTOK : (r + 1) * NTOK],
                            in_=k_r[j * 128 : (j + 1) * 128, :])
                    for tt in range(2):
                        nc.sync.dma_start(
                            out=v_sb[2 * r + tt][:, :, 0:64],
                            in_=v_r[tt * 128 : (tt + 1) * 128])
                for j in range(8):
                    nc.sync.dma_start(out=v_sb[j][:, :, 64:65], in_=onesb.ap()[:, 0:12])

                # --- attention per head ---
                yT = [yp.tile([128, NTOK], BF, name=f"y{j}", tag=f"y{j}") for j in range(KT)]
                for h in range(H):
                    p0 = 64 * (h % 2)
                    q_ap = qkT[h // 2][p0 : p0 + 64, :]
                    psy = ps_av.tile([65, NTOK], F32, name="av", tag="av")
                    for kt in range(8):
                        pss = ps_s.tile([128, NTOK], F32, name="s", tag="s")
                        nc.tensor.matmul(
                            pss,
                            _r(k_sb[h // 2][p0 : p0 + 64, kt * 128 : (kt + 1) * 128]),
                            _r(q_ap), start=True, stop=True)
                        es32 = att.tile([128, NTOK], F32, name="es32", tag="es32")
                        nc.vector.tensor_add(es32, pss, mask_sb[:, kt, :])
                        es = att.tile([128, NTOK], BF, name="es", tag="es")
                        nc.scalar.activation(es, es32, AF.Exp, scale=0.125)
                        nc.tensor.matmul(
                            psy, _r(v_sb[kt][:, h, :]), _r(es),
                            start=(kt == 0), stop=(kt == 7))
                    rec = stat.tile([1, NTOK], F32, name="rec", tag="rec")
                    nc.vector.reciprocal(rec, psy[64:65, :])
                    rb = stat.tile([64, NTOK], F32, name="rb", tag="rb")
                    nc.gpsimd.partition_broadcast(rb, rec)
                    nc.vector.tensor_mul(yT[h // 2][p0 : p0 + 64, :],
                                         psy[0:64, :], rb)

                # --- proj + residual ---
                x2_tiles = []
                for ot in range(KT):
                    wt = wpool.tile([128, KT, 128], BF, name="wp", tag="wp", bufs=2)
                    nc.sync.dma_start(out=wt, in_=wproj[l, ot])
                    ps = ps_mm.tile([128, NTOK], F32, name="mm", tag="mm")
                    for j in range(KT):
                        nc.tensor.matmul(
                            ps, _r(wt[:, j, :]),
                            _r(yT[j]), start=(j == 0), stop=(j == KT - 1))
                    x2 = resid.tile([128, NTOK], F32R, name=f"x{ot}", tag=f"x{ot}")
                    nc.vector.tensor_scalar_add(ps, ps, bias_sb[:, l, 12 + ot : 13 + ot])
                    nc.vector.tensor_add(x2, ps, x_tiles[ot])
                    x2_tiles.append(x2)

                # --- MLP ---
                ln2 = layernorm(x2_tiles, scl[:, 4 * l + 2, :], scl[:, 4 * l + 3, :])
                h_sb = []
                for ot in range(24):
                    wt = wpool.tile([128, KT, 128], BF, name="wf", tag="wf", bufs=2)
                    nc.sync.dma_start(out=wt, in_=wfc[l, ot])
                    ps = ps_mm.tile([128, NTOK], F32, name="mm", tag="mm")
                    for j in range(KT):
                        nc.tensor.matmul(
                            ps, _r(wt[:, j, :]),
                            _r(ln2[j]), start=(j == 0), stop=(j == KT - 1))
                    hs = hp.tile([128, NTOK], BF, name=f"h{ot}", tag=f"h{ot}")
                    nc.scalar.activation(hs, ps, AF.Gelu_apprx_tanh,
                                         bias=bias_sb[:, l, 18 + ot : 19 + ot])
                    h_sb.append(hs)
                x3_tiles = []
                for ot in range(KT):
                    wt = w2pool.tile([128, 24, 128], BF, name="w2", tag="w2", bufs=2)
                    nc.sync.dma_start(out=wt, in_=wfc2[l, ot])
                    ps = ps_mm.tile([128, NTOK], F32, name="mm", tag="mm")
                    for j in range(24):
                        nc.tensor.matmul(
                            ps, _r(wt[:, j, :]),
                            _r(h_sb[j]), start=(j == 0), stop=(j == 23))
                    x3 = resid.tile([128, NTOK], F32R, name=f"x{ot}", tag=f"x{ot}")
                    nc.vector.tensor_scalar_add(ps, ps, bias_sb[:, l, 42 + ot : 43 + ot])
                    nc.vector.tensor_add(x3, ps, x2_tiles[ot])
                    x3_tiles.append(x3)
                x_tiles = x3_tiles

            # --- final LN + AllGather of hidden state ---
            lnf = layernorm(x_tiles, scl[:, 4 * L, :], scl[:, 4 * L + 1, :])
            xf_ap = xf_in.ap().rearrange("(p t) -> p t", p=D)
            for j in range(KT):
                nc.sync.dma_start(out=xf_ap[j * 128 : (j + 1) * 128, :], in_=lnf[j])
            nc.gpsimd.collective_compute(
                "AllGather", ALU.bypass, replica_groups=[list(range(NC))],
                ins=[xf_in.ap()], outs=[xf_out.ap()],
            )
            warm_fill(FILL_FINAL)
            nc.sync.dma_start(out=warm_sink.ap(), in_=warm_out)

        # --- logits: out[t, vshard] = xf.T @ wteT ---
        with ExitStack() as lg:
            xfp = lg.enter_context(tc.tile_pool(name="xfp", bufs=1))
            wtep = lg.enter_context(tc.tile_pool(name="wtep", bufs=2))
            outp = lg.enter_context(tc.tile_pool(name="outp", bufs=4))
            ps_l = lg.enter_context(tc.tile_pool(name="ps_l", bufs=4, space="PSUM"))
            xfo = xf_out.ap()
            xf_sb = [xfp.tile([128, B * T], BF, name=f"xf{j}", tag=f"xf{j}") for j in range(KT)]
            for r in range(NC):
                x_r = xfo[r * KVSZ : (r + 1) * KVSZ].rearrange("(p t) -> p t", p=D)
                for j in range(KT):
                    nc.sync.dma_start(
                        out=xf_sb[j][:, r * NTOK : (r + 1) * NTOK],
                        in_=x_r[j * 128 : (j + 1) * 128, :])
            for v2 in range((VT + 1) // 2):
                w = 1024 if 2 * v2 + 1 < VT else 512
                wt_sb = [wtep.tile([128, 1024], BF, name=f"wte{j}", tag=f"wte{j}") for j in range(KT)]
                for j in range(KT):
                    nc.sync.dma_start(out=wt_sb[j][:, 0:512], in_=wteT[j, 2 * v2])
                    if w == 1024:
                        nc.sync.dma_start(out=wt_sb[j][:, 512:1024], in_=wteT[j, 2 * v2 + 1])
                for tt in range(TT):
                    ps = ps_l.tile([128, 1024], F32, name="lg", tag="lg")
                    for h0 in range(0, w, 512):
                        for j in range(KT):
                            nc.tensor.matmul(
                                ps[:, h0 : h0 + 512],
                                _r(xf_sb[j][:, tt * 128 : (tt + 1) * 128]),
                                _r(wt_sb[j][:, h0 : h0 + 512]),
                                start=(j == 0), stop=(j == KT - 1))
                    ot = outp.tile([128, 1024], BF, name="ob", tag="ob")
                    nc.scalar.copy(ot[:, 0:w], ps[:, 0:w])
                    nc.sync.dma_start(
                        out=out.ap()[tt * 128 : (tt + 1) * 128,
                                     v2 * 1024 : v2 * 1024 + w],
                        in_=ot[:, 0:w])

    nc.compile()
    return nc


def prep_inputs(idx, wte, wpe, ln1_s, ln1_b, attn_w, attn_b, proj_w, proj_b,
                ln2_s, ln2_b, fc_w, fc_b, fc2_w, fc2_b, lnf_s, lnf_b):
    f = np.float32
    bf = ml_dtypes.bfloat16
    x0 = (wte[idx.reshape(-1)] + np.tile(wpe, (B, 1))).astype(f)  # [2048, 768]
    wte_pad = np.zeros((NC * VSHARD, D), f)
    wte_pad[:V] = wte
    shared = {
        "wqk": np.ascontiguousarray(
            attn_w[:, :, :1536].reshape(L, KT, 128, 12, 128).transpose(0, 3, 2, 1, 4)).astype(bf),
        "wv": np.ascontiguousarray(
            attn_w[:, :, 1536:].reshape(L, KT, 128, 2, 384).transpose(0, 3, 2, 1, 4)).astype(bf),
        "wproj": np.ascontiguousarray(
            proj_w.reshape(L, KT, 128, KT, 128).transpose(0, 3, 2, 1, 4)).astype(bf),
        "wfc": np.ascontiguousarray(
            fc_w.reshape(L, KT, 128, 24, 128).transpose(0, 3, 2, 1, 4)).astype(bf),
        "wfc2": np.ascontiguousarray(
            fc2_w.reshape(L, 24, 128, KT, 128).transpose(0, 3, 2, 1, 4)).astype(bf),
        "b_qkv": np.ascontiguousarray(
            attn_b[:, :1536].reshape(L, 12, 128).transpose(0, 2, 1)).astype(f),
        "b_v": np.ascontiguousarray(attn_b[:, 1536:]).astype(f),
        "b_proj": np.ascontiguousarray(proj_b.reshape(L, KT, 128).transpose(0, 2, 1)).astype(f),
        "b_fc": np.ascontiguousarray(fc_b.reshape(L, 24, 128).transpose(0, 2, 1)).astype(f),
        "b_fc2": np.ascontiguousarray(fc2_b.reshape(L, KT, 128).transpose(0, 2, 1)).astype(f),
        "s_ln1": np.ascontiguousarray(ln1_s.reshape(L, KT, 128).transpose(0, 2, 1)).astype(f),
        "bi_ln1": np.ascontiguousarray(ln1_b.reshape(L, KT, 128).transpose(0, 2, 1)).astype(f),
        "s_ln2": np.ascontiguousarray(ln2_s.reshape(L, KT, 128).transpose(0, 2, 1)).astype(f),
        "bi_ln2": np.ascontiguousarray(ln2_b.reshape(L, KT, 128).transpose(0, 2, 1)).astype(f),
        "s_lnf": np.ascontiguousarray(lnf_s.reshape(KT, 128).T).astype(f),
        "bi_lnf": np.ascontiguousarray(lnf_b.reshape(KT, 128).T).astype(f),
    }
    in_maps = []
    tk = np.arange(T)[:, None]
    for c in range(NC):
        qs = NTOK * (c % 4)
        m = np.where(tk <= qs + np.arange(NTOK)[None, :], 0.0, MASKVAL).astype(f)
        wsh = wte_pad[c * VSHARD : (c + 1) * VSHARD]  # [6656, 768]
        wteT_t = np.ascontiguousarray(
            wsh.T.reshape(KT, 128, VT, 512).transpose(0, 2, 1, 3)).astype(bf)
        im = dict(shared)
        im["onesd"] = np.ones((128, 65), f)
        im["onesb"] = np.ones((128, 512), bf)
        im["x0T"] = np.ascontiguousarray(x0[c * NTOK : (c + 1) * NTOK].T)
        im["mask8"] = m
        im["wteT"] = wteT_t
        in_maps.append(im)
    return in_maps


def kernel(**inputs):
    inputs = {k: np.asarray(v) for k, v in inputs.items()}
    in_maps = prep_inputs(**inputs)
    if "nc" not in _CACHE:
        _CACHE["nc"] = build_nc()
    res = run_bass_kernel_spmd(_CACHE["nc"], in_maps, list(range(NC)))
    shards = [np.asarray(res.results[c]["out"]).astype(np.float32) for c in range(NC)]
    full = np.concatenate(shards, axis=1)[:, :V]
    return np.ascontiguousarray(full.reshape(B, T, V))


# revision 30
# speedup vs baseline: 1.2554x; 1.2554x over previous
"""GPT-2 (6L, D=768, H=12, B=2, T=1024, V=50257) forward pass on 8 trn2 cores.

Sharding: tokens 2048 -> 256/core (cores 0-3 = batch 0, 4-7 = batch 1).
Attention: per-layer AllGather of K/V (bf16) within each 4-core batch group;
every core computes full-kv attention for its 256 queries with an additive
mask (SPMD-uniform work). Logits: vocab-sharded (6656 padded cols/core)
against an AllGathered final hidden state; host concatenates shards.
Matmuls run in bf16 (fp32 PSUM accumulation); the residual stream, LN
statistics and pre-softmax scores stay fp32. During each collective the PE
runs filler matmuls so the HAM clock gate never re-throttles to 1.2 GHz.
"""

import sys
from contextlib import ExitStack

import numpy as np
import ml_dtypes

sys.path.insert(0, "/opt/trn_rl_repo")

import concourse.bass as bass
import concourse.tile as tile
from concourse import bacc, mybir
from concourse.bass_utils import run_bass_kernel_spmd

F32 = mybir.dt.float32
F32R = mybir.dt.float32r
BF = mybir.dt.bfloat16
AF = mybir.ActivationFunctionType
ALU = mybir.AluOpType

L, D, V, B, T, H, HD = 6, 768, 50257, 2, 1024, 12, 64
NTOK = 256           # tokens per core
NC = 8               # cores
KT = D // 128        # 6 feature tiles
VSHARD = 6656        # padded vocab per core (13 * 512); 8*6656 = 53248
VT = VSHARD // 512   # 13
TT = (B * T) // 128  # 16 token tiles of the full sequence
MASKVAL = -240.0     # pre-scale additive mask (-30 after 1/8 scaling)
FILL_LAYER = 190     # warm-filler matmuls issued during each KV AllGather
FILL_FINAL = 280     # and during the final hidden-state AllGather

_CACHE = {}


def _r(x):
    return x


def build_nc(debug=False):
    nc = bacc.Bacc("TRN2", target_bir_lowering=False, debug=False, num_devices=NC)

    # ---- per-core inputs ----
    x0T = nc.dram_tensor("x0T", [D, NTOK], F32R, kind="ExternalInput")
    onesd = nc.dram_tensor("onesd", [128, 65], F32R, kind="ExternalInput")
    onesb = nc.dram_tensor("onesb", [128, 512], BF, kind="ExternalInput")
    mask8 = nc.dram_tensor("mask8", [T, NTOK], BF, kind="ExternalInput")
    wteT = nc.dram_tensor("wteT", [KT, VT, 128, 512], BF, kind="ExternalInput")
    # ---- replicated weights ----
    wqk = nc.dram_tensor("wqk", [L, 12, 128, KT, 128], BF, kind="ExternalInput")
    wv = nc.dram_tensor("wv", [L, 2, 128, KT, 384], BF, kind="ExternalInput")
    wproj = nc.dram_tensor("wproj", [L, KT, 128, KT, 128], BF, kind="ExternalInput")
    wfc = nc.dram_tensor("wfc", [L, 24, 128, KT, 128], BF, kind="ExternalInput")
    wfc2 = nc.dram_tensor("wfc2", [L, KT, 128, 24, 128], BF, kind="ExternalInput")
    b_qkv = nc.dram_tensor("b_qkv", [L, 128, 12], F32, kind="ExternalInput")
    b_v = nc.dram_tensor("b_v", [L, 768], F32, kind="ExternalInput")
    b_proj = nc.dram_tensor("b_proj", [L, 128, KT], F32, kind="ExternalInput")
    b_fc = nc.dram_tensor("b_fc", [L, 128, 24], F32, kind="ExternalInput")
    b_fc2 = nc.dram_tensor("b_fc2", [L, 128, KT], F32, kind="ExternalInput")
    s_ln1 = nc.dram_tensor("s_ln1", [L, 128, KT], F32, kind="ExternalInput")
    bi_ln1 = nc.dram_tensor("bi_ln1", [L, 128, KT], F32, kind="ExternalInput")
    s_ln2 = nc.dram_tensor("s_ln2", [L, 128, KT], F32, kind="ExternalInput")
    bi_ln2 = nc.dram_tensor("bi_ln2", [L, 128, KT], F32, kind="ExternalInput")
    s_lnf = nc.dram_tensor("s_lnf", [128, KT], F32, kind="ExternalInput")
    bi_lnf = nc.dram_tensor("bi_lnf", [128, KT], F32, kind="ExternalInput")
    # ---- outputs ----
    out = nc.dram_tensor("out", [B * T, VSHARD], BF, kind="ExternalOutput")
    warm_sink = nc.dram_tensor("warm_sink", [1, 4], F32, kind="ExternalOutput")

    # ---- collective bounce buffers (bf16 payloads) ----
    KVSZ = D * NTOK  # 196608 elems for k (and again for v)
    kv_in = nc.dram_tensor("kv_in", [2 * KVSZ], BF)
    kv_out = nc.dram_tensor("kv_out", [8 * KVSZ], BF)
    xf_in = nc.dram_tensor("xf_in", [KVSZ], BF)
    xf_out = nc.dram_tensor("xf_out", [NC * KVSZ], BF, addr_space="Shared")
    kv_groups = [[0, 1, 2, 3], [4, 5, 6, 7]]

    with tile.TileContext(nc) as tc, ExitStack() as ctx:
        const = ctx.enter_context(tc.tile_pool(name="const", bufs=1))
        ones = const.tile([128, 1], F32R)
        nc.sync.dma_start(out=ones, in_=onesd[:, 0:1])
        eps = const.tile([1, 1], F32)
        nc.vector.memset(eps, 1e-5)
        warm_a = const.tile([128, 128], BF)
        nc.sync.dma_start(out=warm_a, in_=onesb.ap()[:, 0:128])
        warm_b = const.tile([128, 512], BF)
        nc.sync.dma_start(out=warm_b, in_=onesb.ap())
        mask_sb = const.tile([128, 8, NTOK], BF)
        nc.sync.dma_start(
            out=mask_sb,
            in_=mask8.ap().rearrange("(n p) t -> p n t", p=128),
        )
        scl = const.tile([128, 4 * L + 2, KT], F32)  # ln scales/biases
        for l in range(L):
            nc.sync.dma_start(out=scl[:, 4 * l + 0, :], in_=s_ln1[l])
            nc.sync.dma_start(out=scl[:, 4 * l + 1, :], in_=bi_ln1[l])
            nc.sync.dma_start(out=scl[:, 4 * l + 2, :], in_=s_ln2[l])
            nc.sync.dma_start(out=scl[:, 4 * l + 3, :], in_=bi_ln2[l])
        nc.sync.dma_start(out=scl[:, 4 * L + 0, :], in_=s_lnf.ap())
        nc.sync.dma_start(out=scl[:, 4 * L + 1, :], in_=bi_lnf.ap())
        bias_sb = const.tile([128, L, 12 + KT + 24 + KT], F32)
        for l in range(L):
            nc.sync.dma_start(out=bias_sb[:, l, 0:12], in_=b_qkv[l])
            nc.sync.dma_start(out=bias_sb[:, l, 12 : 12 + KT], in_=b_proj[l])
            nc.sync.dma_start(out=bias_sb[:, l, 18:42], in_=b_fc[l])
            nc.sync.dma_start(out=bias_sb[:, l, 42:48], in_=b_fc2[l])

        with ExitStack() as body:
            resid = body.enter_context(tc.tile_pool(name="resid", bufs=2))
            lnp = body.enter_context(tc.tile_pool(name="lnp", bufs=1))
            qkvp = body.enter_context(tc.tile_pool(name="qkvp", bufs=1))
            kvp = body.enter_context(tc.tile_pool(name="kvp", bufs=1))
            wpool = body.enter_context(tc.tile_pool(name="wpool", bufs=1))
            w2pool = body.enter_context(tc.tile_pool(name="w2pool", bufs=2))
            att = body.enter_context(tc.tile_pool(name="att", bufs=4))
            yp = body.enter_context(tc.tile_pool(name="yp", bufs=1))
            hp = body.enter_context(tc.tile_pool(name="hp", bufs=1))
            stat = body.enter_context(tc.tile_pool(name="stat", bufs=1))
            ps_mm = body.enter_context(tc.tile_pool(name="ps_mm", bufs=2, space="PSUM"))
            ps_s = body.enter_context(tc.tile_pool(name="ps_s", bufs=2, space="PSUM"))
            ps_av = body.enter_context(tc.tile_pool(name="ps_av", bufs=2, space="PSUM"))
            ps_st = body.enter_context(tc.tile_pool(name="ps_st", bufs=1, space="PSUM"))

            # warm-filler scratch: reuses the ps_st pool so PSUM stays at 8 banks
            ps_warm = ps_st.tile([128, 512], F32, name="warm", tag="warm")
            warm_out = stat.tile([1, 4], F32, name="warm_out", tag="warm_out")

            def warm_fill(n):
                for _ in range(n):
                    nc.tensor.matmul(ps_warm, _r(warm_a), _r(warm_b),
                                     start=True, stop=True)
                # liveness anchor: keep the fillers from being dead-code
                # eliminated (warm_out is DMA'd to warm_sink at the end)
                nc.scalar.copy(warm_out, ps_warm[0:1, 0:4])

            x_tiles = []
            for j in range(KT):
                xt = resid.tile([128, NTOK], F32R, name=f"x{j}", tag=f"x{j}")
                nc.sync.dma_start(out=xt, in_=x0T[j * 128 : (j + 1) * 128, :])
                x_tiles.append(xt)

            def layernorm(xs, s_col, b_col):
                ln_ps = ps_st.tile([1, 2, NTOK], F32, name="ln_ps", tag="ln_ps")
                sums = ln_ps[:, 0, :]
                sumq = ln_ps[:, 1, :]
                sq = []
                for j in range(KT):
                    sqt = lnp.tile([128, NTOK], F32R, name=f"sq{j}", tag=f"sq{j}")
                    nc.vector.tensor_mul(sqt, xs[j], xs[j])
                    sq.append(sqt)
                for j in range(KT):
                    nc.tensor.matmul(sums, _r(ones), _r(xs[j]),
                                     start=(j == 0), stop=(j == KT - 1))
                for j in range(KT):
                    nc.tensor.matmul(sumq, _r(ones), _r(sq[j]),
                                     start=(j == 0), stop=(j == KT - 1))
                mu_t = stat.tile([1, NTOK], F32, name="mu_t", tag="mu_t")
                rs_t = stat.tile([1, NTOK], F32, name="rs_t", tag="rs_t")
                mu = mu_t[0:1, :]
                rs = rs_t[0:1, :]
                nc.scalar.mul(mu, sums, 1.0 / D)
                musq = stat.tile([1, NTOK], F32, name="musq", tag="musq")
                nc.vector.tensor_mul(musq, mu, mu)
                var = stat.tile([1, NTOK], F32, name="var", tag="var")
                nc.vector.tensor_scalar(out=var, in0=sumq, scalar1=1.0 / D,
                                        scalar2=None, op0=ALU.mult)
                nc.vector.tensor_sub(var, var, musq)
                # rsqrt via ln+exp: keeps ACT on the natural_log_exp table
                # set (shared with attention's Exp) — avoids ~2.7us
                # sqrt-set table reloads every layer
                nc.scalar.activation(var, var, AF.Ln, bias=eps)
                nc.scalar.activation(rs, var, AF.Exp, scale=-0.5)
                bc = stat.tile([128, 2, NTOK], F32, name="bc", tag="bc")
                nc.gpsimd.partition_broadcast(bc[:, 0, :], mu)
                nc.gpsimd.partition_broadcast(bc[:, 1, :], rs)
                outs = []
                for j in range(KT):
                    tmp = lnp.tile([128, NTOK], BF, name=f"lt{j}", tag=f"lt{j}")
                    nc.vector.tensor_sub(tmp, xs[j], bc[:, 0, :])
                    nc.vector.tensor_mul(tmp, tmp, bc[:, 1, :])
                    ot = lnp.tile([128, NTOK], BF, name=f"ln{j}", tag=f"ln{j}")
                    nc.vector.tensor_scalar(out=ot, in0=tmp,
                                            scalar1=s_col[:, j : j + 1],
                                            scalar2=b_col[:, j : j + 1],
                                            op0=ALU.mult, op1=ALU.add)
                    outs.append(ot)
                return outs

            kv_in_ap = kv_in.ap()
            k_in = kv_in_ap[0:KVSZ].rearrange("(p t) -> p t", p=D)
            v_in = kv_in_ap[KVSZ:].rearrange("(t d) -> t d", t=NTOK)

            # persistent gathered-KV tiles; the softmax-denominator ones
            # column is written once and survives all layers (bufs=1 slots)
            k_all = kvp.tile([128, KT, T], BF, name="k_all", tag="k_all")
            v_all = [kvp.tile([128, 2, 12, 65], BF, name=f"v{r}", tag=f"v{r}")
                     for r in range(4)]
            for r in range(4):
                nc.sync.dma_start(
                    out=v_all[r][:, :, :, 64:65],
                    in_=onesb.ap()[:, 0:24].rearrange("p (a h) -> p a h", a=2))

            for l in range(L):
                sc = scl[:, 4 * l + 0, :]
                bc1 = scl[:, 4 * l + 1, :]
                ln1 = layernorm(x_tiles, sc, bc1)

                # --- QK part of QKV: feature-major out [1536, 256] ---
                qkT = []
                wq_all = wpool.tile([128, 12, KT, 128], BF, name="wqk", tag="wqk")
                nc.sync.dma_start(out=wq_all, in_=wqk[l].rearrange("o p j q -> p o j q"))
                for ot in range(12):
                    ps = ps_mm.tile([128, NTOK], F32, name="mm", tag="mm")
                    for j in range(KT):
                        nc.tensor.matmul(
                            ps, _r(wq_all[:, ot, j, :]),
                            _r(ln1[j]), start=(j == 0), stop=(j == KT - 1))
                    sb = qkvp.tile([128, NTOK], BF, name=f"qk{ot}", tag=f"qk{ot}")
                    nc.vector.tensor_scalar_add(sb, ps, bias_sb[:, l, ot : ot + 1])
                    qkT.append(sb)
                    if ot >= 6:  # k tiles -> collective input
                        nc.sync.dma_start(
                            out=k_in[(ot - 6) * 128 : (ot - 5) * 128, :], in_=sb)

                # --- V part: token-major out [256, 768] ---
                bv_sb = wpool.tile([128, 768], F32, name="bv", tag="bv")
                bvl = b_v.ap()[l]
                nc.sync.dma_start(
                    out=bv_sb,
                    in_=bass.AP(tensor=bvl.tensor, offset=bvl.offset,
                                ap=[[0, 128]] + list(bvl.ap)),
                )
                vloc = [qkvp.tile([128, 768], BF, name=f"vloc{tt}", tag=f"vloc{tt}")
                        for tt in range(2)]
                wv_all = wpool.tile([128, 2, KT, 384], BF, name="wvt", tag="wvt")
                nc.sync.dma_start(out=wv_all, in_=wv[l].rearrange("o p j q -> p o j q"))
                for oh in range(2):
                    for tt in range(2):
                        ps = ps_mm.tile([128, 384], F32, name="mmv", tag="mm")
                        for j in range(KT):
                            nc.tensor.matmul(
                                ps, _r(ln1[j][:, tt * 128 : (tt + 1) * 128]),
                                _r(wv_all[:, oh, j, :]),
                                start=(j == 0), stop=(j == KT - 1))
                        nc.vector.tensor_add(
                            vloc[tt][:, oh * 384 : (oh + 1) * 384], ps,
                            bv_sb[:, oh * 384 : (oh + 1) * 384])
                for tt in range(2):
                    nc.sync.dma_start(
                        out=v_in[tt * 128 : (tt + 1) * 128, :], in_=vloc[tt])

                nc.gpsimd.collective_compute(
                    "AllGather", ALU.bypass, replica_groups=kv_groups,
                    ins=[kv_in.ap()], outs=[kv_out.ap()[0 : 4 * 2 * KVSZ]],
                )
                warm_fill(FILL_LAYER)

                # --- load gathered K (feature-major [768, 1024]) and V ---
                kvo = kv_out.ap()
                for r in range(4):
                    base = r * 2 * KVSZ
                    k_r = kvo[base : base + KVSZ].rearrange(
                        "(j p t) -> p j t", p=128, t=NTOK)
                    nc.sync.dma_start(
                        out=k_all[:, :, r * NTOK : (r + 1) * NTOK], in_=k_r)
                    for a in range(2):
                        v_r = kvo[base + KVSZ + a * KVSZ // 2
                                  : base + KVSZ + (a + 1) * KVSZ // 2].rearrange(
                            "(p h d) -> p h d", p=128, h=12)
                        nc.sync.dma_start(out=v_all[r][:, a, :, 0:64], in_=v_r)

                # --- attention per head (kv blocks paired into 512-wide ops) ---
                yT = [yp.tile([128, NTOK], BF, name=f"y{j}", tag=f"y{j}") for j in range(KT)]
                for h in range(H):
                    p0 = 64 * (h % 2)
                    q_ap = qkT[h // 2][p0 : p0 + 64, :]
                    psy = ps_av.tile([65, NTOK], F32, name="av", tag="av")
                    for kp in range(4):
                        pss = ps_s.tile([128, 2, NTOK], F32, name="s", tag="s")
                        for half in range(2):
                            kt = 2 * kp + half
                            nc.tensor.matmul(
                                pss[:, half, :],
                                _r(k_all[p0 : p0 + 64, h // 2,
                                         kt * 128 : (kt + 1) * 128]),
                                _r(q_ap), start=True, stop=True)
                        es = att.tile([128, 2, NTOK], BF, name="es", tag="es")
                        nc.vector.tensor_add(es, pss, mask_sb[:, 2 * kp : 2 * kp + 2, :])
                        nc.scalar.activation(es, es, AF.Exp, scale=0.125)
                        for half in range(2):
                            kt = 2 * kp + half
                            nc.tensor.matmul(
                                psy, _r(v_all[kp][:, half, h, :]),
                                _r(es[:, half, :]),
                                start=(kt == 0), stop=(kt == 7))
                    rec = stat.tile([1, NTOK], F32, name="rec", tag="rec")
                    nc.vector.reciprocal(rec, psy[64:65, :])
                    rb = stat.tile([64, NTOK], F32, name="rb", tag="rb")
                    nc.gpsimd.partition_broadcast(rb, rec)
                    nc.vector.tensor_mul(yT[h // 2][p0 : p0 + 64, :],
                                         psy[0:64, :], rb)

                # --- proj + residual ---
                x2_tiles = []
                wp_all = wpool.tile([128, KT, KT, 128], BF, name="wp", tag="wp")
                nc.sync.dma_start(out=wp_all, in_=wproj[l].rearrange("o p j q -> p o j q"))
                for ot in range(KT):
                    ps = ps_mm.tile([128, NTOK], F32, name="mm", tag="mm")
                    for j in range(KT):
                        nc.tensor.matmul(
                            ps, _r(wp_all[:, ot, j, :]),
                            _r(yT[j]), start=(j == 0), stop=(j == KT - 1))
                    x2 = resid.tile([128, NTOK], F32R, name=f"x{ot}", tag=f"x{ot}")
                    nc.vector.tensor_scalar_add(ps, ps, bias_sb[:, l, 12 + ot : 13 + ot])
                    nc.vector.tensor_add(x2, ps, x_tiles[ot])
                    x2_tiles.append(x2)

                # --- MLP ---
                ln2 = layernorm(x2_tiles, scl[:, 4 * l + 2, :], scl[:, 4 * l + 3, :])
                h_sb = []
                for g in range(2):
                    wf_g = wpool.tile([128, 12, KT, 128], BF, name="wf", tag="wf",
                                      bufs=2)
                    nc.sync.dma_start(
                        out=wf_g,
                        in_=wfc[l, g * 12 : (g + 1) * 12].rearrange("o p j q -> p o j q"))
                    for oi in range(12):
                        ot = g * 12 + oi
                        ps = ps_mm.tile([128, NTOK], F32, name="mm", tag="mm")
                        for j in range(KT):
                            nc.tensor.matmul(
                                ps, _r(wf_g[:, oi, j, :]),
                                _r(ln2[j]), start=(j == 0), stop=(j == KT - 1))
                        hs = hp.tile([128, NTOK], BF, name=f"h{ot}", tag=f"h{ot}")
                        nc.scalar.activation(hs, ps, AF.Gelu_apprx_tanh,
                                             bias=bias_sb[:, l, 18 + ot : 19 + ot])
                        h_sb.append(hs)
                x3_tiles = []
                for g2 in range(2):
                    w2_g = w2pool.tile([128, 3, 24, 128], BF, name="w2", tag="w2",
                                       bufs=2)
                    nc.sync.dma_start(
                        out=w2_g,
                        in_=wfc2[l, g2 * 3 : (g2 + 1) * 3].rearrange("o p j q -> p o j q"))
                    for oi in range(3):
                        ot = g2 * 3 + oi
                        ps = ps_mm.tile([128, NTOK], F32, name="mm", tag="mm")
                        for j in range(24):
                            nc.tensor.matmul(
                                ps, _r(w2_g[:, oi, j, :]),
                                _r(h_sb[j]), start=(j == 0), stop=(j == 23))
                        x3 = resid.tile([128, NTOK], F32R, name=f"x{ot}", tag=f"x{ot}")
                        nc.vector.tensor_scalar_add(ps, ps, bias_sb[:, l, 42 + ot : 43 + ot])
                        nc.vector.tensor_add(x3, ps, x2_tiles[ot])
                        x3_tiles.append(x3)
                x_tiles = x3_tiles

            # --- final LN + AllGather of hidden state ---
            lnf = layernorm(x_tiles, scl[:, 4 * L, :], scl[:, 4 * L + 1, :])
            xf_ap = xf_in.ap().rearrange("(p t) -> p t", p=D)
            for j in range(KT):
                nc.sync.dma_start(out=xf_ap[j * 128 : (j + 1) * 128, :], in_=lnf[j])
            nc.gpsimd.collective_compute(
                "AllGather", ALU.bypass, replica_groups=[list(range(NC))],
                ins=[xf_in.ap()], outs=[xf_out.ap()],
            )
            warm_fill(FILL_FINAL)
            nc.sync.dma_start(out=warm_sink.ap(), in_=warm_out)

        # --- logits: out[t, vshard] = xf.T @ wteT ---
        with ExitStack() as lg:
            xfp = lg.enter_context(tc.tile_pool(name="xfp", bufs=1))
            wtep = lg.enter_context(tc.tile_pool(name="wtep", bufs=2))
            outp = lg.enter_context(tc.tile_pool(name="outp", bufs=4))
            ps_l = lg.enter_context(tc.tile_pool(name="ps_l", bufs=4, space="PSUM"))
            xfo = xf_out.ap()
            xf_sb = [xfp.tile([128, B * T], BF, name=f"xf{j}", tag=f"xf{j}") for j in range(KT)]
            for r in range(NC):
                x_r = xfo[r * KVSZ : (r + 1) * KVSZ].rearrange("(p t) -> p t", p=D)
                for j in range(KT):
                    nc.sync.dma_start(
                        out=xf_sb[j][:, r * NTOK : (r + 1) * NTOK],
                        in_=x_r[j * 128 : (j + 1) * 128, :])
            for v2 in range((VT + 1) // 2):
                w = 1024 if 2 * v2 + 1 < VT else 512
                wt_sb = [wtep.tile([128, 1024], BF, name=f"wte{j}", tag=f"wte{j}") for j in range(KT)]
                for j in range(KT):
                    nc.sync.dma_start(out=wt_sb[j][:, 0:512], in_=wteT[j, 2 * v2])
                    if w == 1024:
                        nc.sync.dma_start(out=wt_sb[j][:, 512:1024], in_=wteT[j, 2 * v2 + 1])
                for tt in range(TT):
                    ps = ps_l.tile([128, 1024], F32, name="lg", tag="lg")
                    for h0 in range(0, w, 512):
                        for j in range(KT):
                            nc.tensor.matmul(
                                ps[:, h0 : h0 + 512],
                                _r(xf_sb[j][:, tt * 128 : (tt + 1) * 128]),
                                _r(wt_sb[j][:, h0 : h0 + 512]),
                                start=(j == 0), stop=(j == KT - 1))
                    ot = outp.tile([128, 1024], BF, name="ob", tag="ob")
                    nc.scalar.copy(ot[:, 0:w], ps[:, 0:w])
                    nc.sync.dma_start(
                        out=out.ap()[tt * 128 : (tt + 1) * 128,
                                     v2 * 1024 : v2 * 1024 + w],
                        in_=ot[:, 0:w])

    nc.compile()
    return nc


def prep_inputs(idx, wte, wpe, ln1_s, ln1_b, attn_w, attn_b, proj_w, proj_b,
                ln2_s, ln2_b, fc_w, fc_b, fc2_w, fc2_b, lnf_s, lnf_b):
    f = np.float32
    bf = ml_dtypes.bfloat16
    x0 = (wte[idx.reshape(-1)] + np.tile(wpe, (B, 1))).astype(f)  # [2048, 768]
    wte_pad = np.zeros((NC * VSHARD, D), f)
    wte_pad[:V] = wte
    shared = {
        "wqk": np.ascontiguousarray(
            attn_w[:, :, :1536].reshape(L, KT, 128, 12, 128).transpose(0, 3, 2, 1, 4)).astype(bf),
        "wv": np.ascontiguousarray(
            attn_w[:, :, 1536:].reshape(L, KT, 128, 2, 384).transpose(0, 3, 2, 1, 4)).astype(bf),
        "wproj": np.ascontiguousarray(
            proj_w.reshape(L, KT, 128, KT, 128).transpose(0, 3, 2, 1, 4)).astype(bf),
        "wfc": np.ascontiguousarray(
            fc_w.reshape(L, KT, 128, 24, 128).transpose(0, 3, 2, 1, 4)).astype(bf),
        "wfc2": np.ascontiguousarray(
            fc2_w.reshape(L, 24, 128, KT, 128).transpose(0, 3, 2, 1, 4)).astype(bf),
        "b_qkv": np.ascontiguousarray(
            attn_b[:, :1536].reshape(L, 12, 128).transpose(0, 2, 1)).astype(f),
        "b_v": np.ascontiguousarray(attn_b[:, 1536:]).astype(f),
        "b_proj": np.ascontiguousarray(proj_b.reshape(L, KT, 128).transpose(0, 2, 1)).astype(f),
        "b_fc": np.ascontiguousarray(fc_b.reshape(L, 24, 128).transpose(0, 2, 1)).astype(f),
        "b_fc2": np.ascontiguousarray(fc2_b.reshape(L, KT, 128).transpose(0, 2, 1)).astype(f),
        "s_ln1": np.ascontiguousarray(ln1_s.reshape(L, KT, 128).transpose(0, 2, 1)).astype(f),
        "bi_ln1": np.ascontiguousarray(ln1_b.reshape(L, KT, 128).transpose(0, 2, 1)).astype(f),
        "s_ln2": np.ascontiguousarray(ln2_s.reshape(L, KT, 128).transpose(0, 2, 1)).astype(f),
        "bi_ln2": np.ascontiguousarray(ln2_b.reshape(L, KT, 128).transpose(0, 2, 1)).astype(f),
        "s_lnf": np.ascontiguousarray(lnf_s.reshape(KT, 128).T).astype(f),
        "bi_lnf": np.ascontiguousarray(lnf_b.reshape(KT, 128).T).astype(f),
    }
    in_maps = []
    tk = np.arange(T)[:, None]
    for c in range(NC):
        qs = NTOK * (c % 4)
        m = np.where(tk <= qs + np.arange(NTOK)[None, :], 0.0, MASKVAL).astype(f)
        wsh = wte_pad[c * VSHARD : (c + 1) * VSHARD]  # [6656, 768]
        wteT_t = np.ascontiguousarray(
            wsh.T.reshape(KT, 128, VT, 512).transpose(0, 2, 1, 3)).astype(bf)
        im = dict(shared)
        im["onesd"] = np.ones((128, 65), f)
        im["onesb"] = np.ones((128, 512), bf)
        im["x0T"] = np.ascontiguousarray(x0[c * NTOK : (c + 1) * NTOK].T)
        im["mask8"] = m.astype(bf)
        im["wteT"] = wteT_t
        in_maps.append(im)
    return in_maps


def kernel(**inputs):
    inputs = {k: np.asarray(v) for k, v in inputs.items()}
    in_maps = prep_inputs(**inputs)
    if "nc" not in _CACHE:
        _CACHE["nc"] = build_nc()
    res = run_bass_kernel_spmd(_CACHE["nc"], in_maps, list(range(NC)))
    shards = [np.asarray(res.results[c]["out"]).astype(np.float32) for c in range(NC)]
    full = np.concatenate(shards, axis=1)[:, :V]
    return np.ascontiguousarray(full.reshape(B, T, V))


# revision 45
# speedup vs baseline: 1.3061x; 1.0404x over previous
"""GPT-2 (6L, D=768, H=12, B=2, T=1024, V=50257) forward pass on 8 trn2 cores.

Sharding: tokens 2048 -> 256/core (cores 0-3 = batch 0, 4-7 = batch 1).
Attention: per-layer AllGather of K/V (bf16) within each 4-core batch group;
every core computes full-kv attention for its 256 queries with an additive
mask (SPMD-uniform work). Logits: vocab-sharded (6656 padded cols/core)
against an AllGathered final hidden state; host concatenates shards.
Matmuls run in bf16 (fp32 PSUM accumulation); the residual stream, LN
statistics and pre-softmax scores stay fp32. During each collective the PE
runs filler matmuls so the HAM clock gate never re-throttles to 1.2 GHz.
"""

import sys
from contextlib import ExitStack

import numpy as np
import ml_dtypes

sys.path.insert(0, "/opt/trn_rl_repo")

import concourse.bass as bass
import concourse.tile as tile
from concourse import bacc, mybir
from concourse.bass_utils import run_bass_kernel_spmd

F32 = mybir.dt.float32
F32R = mybir.dt.float32r
BF = mybir.dt.bfloat16
AF = mybir.ActivationFunctionType
ALU = mybir.AluOpType

L, D, V, B, T, H, HD = 6, 768, 50257, 2, 1024, 12, 64
NTOK = 256           # tokens per core
NC = 8               # cores
KT = D // 128        # 6 feature tiles
VSHARD = 6656        # padded vocab per core (13 * 512); 8*6656 = 53248
VT = VSHARD // 512   # 13
TT = (B * T) // 128  # 16 token tiles of the full sequence
MASKVAL = -240.0     # pre-scale additive mask (-30 after 1/8 scaling)
FILL_LAYER = 230     # warm-filler matmuls issued during each KV AllGather
FILL_FINAL = 280     # and during the final hidden-state AllGather

_CACHE = {}


def _r(x):
    return x


def build_nc(debug=False):
    nc = bacc.Bacc("TRN2", target_bir_lowering=False, debug=False, num_devices=NC)

    # ---- per-core inputs ----
    x0T = nc.dram_tensor("x0T", [D, NTOK], F32R, kind="ExternalInput")
    onesd = nc.dram_tensor("onesd", [128, 65], F32R, kind="ExternalInput")
    onesb = nc.dram_tensor("onesb", [128, 512], BF, kind="ExternalInput")
    ident = nc.dram_tensor("ident", [128, 128], BF, kind="ExternalInput")
    mask8 = nc.dram_tensor("mask8", [T, NTOK], BF, kind="ExternalInput")
    wteT = nc.dram_tensor("wteT", [KT, VT, 128, 512], BF, kind="ExternalInput")
    # ---- replicated weights ----
    wqk = nc.dram_tensor("wqk", [L, 12, 128, KT, 128], BF, kind="ExternalInput")
    wv = nc.dram_tensor("wv", [L, 2, 128, KT, 384], BF, kind="ExternalInput")
    wproj = nc.dram_tensor("wproj", [L, KT, 128, KT, 128], BF, kind="ExternalInput")
    wfc = nc.dram_tensor("wfc", [L, 24, 128, KT, 128], BF, kind="ExternalInput")
    wfc2 = nc.dram_tensor("wfc2", [L, KT, 128, 24, 128], BF, kind="ExternalInput")
    b_qkv = nc.dram_tensor("b_qkv", [L, 128, 12], F32, kind="ExternalInput")
    b_v = nc.dram_tensor("b_v", [L, 768], F32, kind="ExternalInput")
    b_proj = nc.dram_tensor("b_proj", [L, 128, KT], F32, kind="ExternalInput")
    b_fc = nc.dram_tensor("b_fc", [L, 128, 24], F32, kind="ExternalInput")
    b_fc2 = nc.dram_tensor("b_fc2", [L, 128, KT], F32, kind="ExternalInput")
    s_ln1 = nc.dram_tensor("s_ln1", [L, 128, KT], F32, kind="ExternalInput")
    bi_ln1 = nc.dram_tensor("bi_ln1", [L, 128, KT], F32, kind="ExternalInput")
    s_ln2 = nc.dram_tensor("s_ln2", [L, 128, KT], F32, kind="ExternalInput")
    bi_ln2 = nc.dram_tensor("bi_ln2", [L, 128, KT], F32, kind="ExternalInput")
    s_lnf = nc.dram_tensor("s_lnf", [128, KT], F32, kind="ExternalInput")
    bi_lnf = nc.dram_tensor("bi_lnf", [128, KT], F32, kind="ExternalInput")
    # ---- outputs ----
    out = nc.dram_tensor("out", [B * T, VSHARD], BF, kind="ExternalOutput")
    warm_sink = nc.dram_tensor("warm_sink", [1, 4], F32, kind="ExternalOutput")

    # ---- collective bounce buffers (bf16 payloads) ----
    KVSZ = D * NTOK  # 196608 elems for k (and again for v)
    kv_in = nc.dram_tensor("kv_in", [2 * KVSZ], BF)
    kv_out = nc.dram_tensor("kv_out", [8 * KVSZ], BF)
    xf_in = nc.dram_tensor("xf_in", [KVSZ], BF)
    xf_out = nc.dram_tensor("xf_out", [NC * KVSZ], BF, addr_space="Shared")
    kv_groups = [[0, 1, 2, 3], [4, 5, 6, 7]]

    with tile.TileContext(nc) as tc, ExitStack() as ctx:
        const = ctx.enter_context(tc.tile_pool(name="const", bufs=1))
        ones = const.tile([128, 1], F32R)
        nc.sync.dma_start(out=ones, in_=onesd[:, 0:1])
        eps = const.tile([1, 1], F32)
        nc.vector.memset(eps, 1e-5)
        warm_a = const.tile([128, 128], BF)
        nc.sync.dma_start(out=warm_a, in_=onesb.ap()[:, 0:128])
        warm_b = const.tile([128, 512], BF)
        nc.sync.dma_start(out=warm_b, in_=onesb.ap())
        ident_sb = const.tile([128, 128], BF)
        nc.sync.dma_start(out=ident_sb, in_=ident.ap())
        mask_sb = const.tile([128, 8, NTOK], BF)
        nc.sync.dma_start(
            out=mask_sb,
            in_=mask8.ap().rearrange("(n p) t -> p n t", p=128),
        )
        scl = const.tile([128, 2, KT], F32)  # final-LN scale/bias (others folded)
        nc.sync.dma_start(out=scl[:, 0, :], in_=s_lnf.ap())
        nc.sync.dma_start(out=scl[:, 1, :], in_=bi_lnf.ap())
        bias_sb = const.tile([128, L, 12 + KT + 24 + KT], F32)
        for l in range(L):
            nc.sync.dma_start(out=bias_sb[:, l, 0:12], in_=b_qkv[l])
            nc.sync.dma_start(out=bias_sb[:, l, 12 : 12 + KT], in_=b_proj[l])
            nc.sync.dma_start(out=bias_sb[:, l, 18:42], in_=b_fc[l])
            nc.sync.dma_start(out=bias_sb[:, l, 42:48], in_=b_fc2[l])

        with ExitStack() as body:
            resid = body.enter_context(tc.tile_pool(name="resid", bufs=2))
            lnp = body.enter_context(tc.tile_pool(name="lnp", bufs=1))
            qkvp = body.enter_context(tc.tile_pool(name="qkvp", bufs=1))
            kvp = body.enter_context(tc.tile_pool(name="kvp", bufs=1))
            wpool = body.enter_context(tc.tile_pool(name="wpool", bufs=1))
            w2pool = body.enter_context(tc.tile_pool(name="w2pool", bufs=2))
            att = body.enter_context(tc.tile_pool(name="att", bufs=4))
            yp = body.enter_context(tc.tile_pool(name="yp", bufs=1))
            hp = body.enter_context(tc.tile_pool(name="hp", bufs=1))
            stat = body.enter_context(tc.tile_pool(name="stat", bufs=1))
            ps_mm = body.enter_context(tc.tile_pool(name="ps_mm", bufs=2, space="PSUM"))
            ps_s = body.enter_context(tc.tile_pool(name="ps_s", bufs=2, space="PSUM"))
            ps_av = body.enter_context(tc.tile_pool(name="ps_av", bufs=2, space="PSUM"))
            ps_st = body.enter_context(tc.tile_pool(name="ps_st", bufs=1, space="PSUM"))

            # warm-filler scratch: reuses the ps_st pool so PSUM stays at 8 banks
            ps_warm = ps_st.tile([128, 512], F32, name="warm", tag="warm")
            warm_out = stat.tile([1, 4], F32, name="warm_out", tag="warm_out")

            def warm_fill(n):
                for _ in range(n):
                    nc.tensor.matmul(ps_warm, _r(warm_a), _r(warm_b),
                                     start=True, stop=True)
                # liveness anchor: keep the fillers from being dead-code
                # eliminated (warm_out is DMA'd to warm_sink at the end)
                nc.scalar.copy(warm_out, ps_warm[0:1, 0:4])

            x_tiles = []
            for j in range(KT):
                xt = resid.tile([128, NTOK], F32R, name=f"x{j}", tag=f"x{j}")
                nc.sync.dma_start(out=xt, in_=x0T[j * 128 : (j + 1) * 128, :])
                x_tiles.append(xt)

            def layernorm(xs, s_col, b_col, folded=False, sq_pre=None):
                ln_ps = ps_st.tile([1, 2, NTOK], F32, name="ln_ps", tag="ln_ps")
                sums = ln_ps[:, 0, :]
                sumq = ln_ps[:, 1, :]
                if sq_pre is not None:
                    sq = sq_pre
                else:
                    sq = []
                    for j in range(KT):
                        sqt = lnp.tile([128, NTOK], F32R, name=f"sq{j}", tag=f"sq{j}")
                        nc.vector.tensor_mul(sqt, xs[j], xs[j])
                        sq.append(sqt)
                for j in range(KT):
                    nc.tensor.matmul(sums, _r(ones), _r(xs[j]),
                                     start=(j == 0), stop=(j == KT - 1))
                for j in range(KT):
                    nc.tensor.matmul(sumq, _r(ones), _r(sq[j]),
                                     start=(j == 0), stop=(j == KT - 1))
                mu_t = stat.tile([1, NTOK], F32, name="mu_t", tag="mu_t")
                rs_t = stat.tile([1, NTOK], F32, name="rs_t", tag="rs_t")
                mu = mu_t[0:1, :]
                rs = rs_t[0:1, :]
                nc.scalar.mul(mu, sums, 1.0 / D)
                musq = stat.tile([1, NTOK], F32, name="musq", tag="musq")
                nc.vector.tensor_mul(musq, mu, mu)
                var = stat.tile([1, NTOK], F32, name="var", tag="var")
                nc.vector.tensor_scalar(out=var, in0=sumq, scalar1=1.0 / D,
                                        scalar2=None, op0=ALU.mult)
                nc.vector.tensor_sub(var, var, musq)
                # rsqrt via ln+exp: keeps ACT on the natural_log_exp table
                # set (shared with attention's Exp) — avoids ~2.7us
                # sqrt-set table reloads every layer
                nc.scalar.activation(var, var, AF.Ln, bias=eps)
                nc.scalar.activation(rs, var, AF.Exp, scale=-0.5)
                bc = stat.tile([128, 2, NTOK], F32, name="bc", tag="bc")
                nc.gpsimd.partition_broadcast(bc[:, 0, :], mu)
                nc.gpsimd.partition_broadcast(bc[:, 1, :], rs)
                outs = []
                for j in range(KT):
                    ot = lnp.tile([128, NTOK], BF, name=f"ln{j}", tag=f"ln{j}")
                    nc.vector.tensor_sub(ot, xs[j], bc[:, 0, :])
                    nc.vector.tensor_mul(ot, ot, bc[:, 1, :])
                    if not folded:
                        # lnf scale/bias can't be folded into the tied wte
                        nc.vector.tensor_scalar(out=ot, in0=ot,
                                                scalar1=s_col[:, j : j + 1],
                                                scalar2=b_col[:, j : j + 1],
                                                op0=ALU.mult, op1=ALU.add)
                    outs.append(ot)
                return outs

            kv_in_ap = kv_in.ap()
            k_in = kv_in_ap[0:KVSZ].rearrange("(p t) -> p t", p=D)
            v_in = kv_in_ap[KVSZ:].rearrange("(t d) -> t d", t=NTOK)

            # persistent gathered-KV tiles; the softmax-denominator ones
            # column is written once and survives all layers (bufs=1 slots)
            k_all = kvp.tile([128, KT, T], BF, name="k_all", tag="k_all")
            v_all = [kvp.tile([128, 2, 12, 65], BF, name=f"v{r}", tag=f"v{r}")
                     for r in range(4)]
            for r in range(4):
                nc.sync.dma_start(
                    out=v_all[r][:, :, :, 64:65],
                    in_=onesb.ap()[:, 0:24].rearrange("p (a h) -> p a h", a=2))

            sq_next = None
            for l in range(L):
                ln1 = layernorm(x_tiles, None, None, folded=True, sq_pre=sq_next)

                # --- QK part of QKV: feature-major out [1536, 256] ---
                qkT = []
                wq_all = wpool.tile([128, 12, KT, 128], BF, name="wqk", tag="wqk")
                nc.sync.dma_start(out=wq_all, in_=wqk[l].rearrange("o p j q -> p o j q"))
                for ot in range(12):
                    ps = ps_mm.tile([128, NTOK], F32, name="mm", tag="mm")
                    for j in range(KT):
                        nc.tensor.matmul(
                            ps, _r(wq_all[:, ot, j, :]),
                            _r(ln1[j]), start=(j == 0), stop=(j == KT - 1))
                    sb = qkvp.tile([128, NTOK], BF, name=f"qk{ot}", tag=f"qk{ot}")
                    nc.vector.tensor_scalar_add(sb, ps, bias_sb[:, l, ot : ot + 1])
                    qkT.append(sb)
                    if ot >= 6:  # k tiles -> collective input
                        nc.sync.dma_start(
                            out=k_in[(ot - 6) * 128 : (ot - 5) * 128, :], in_=sb)

                # --- V part: token-major out [256, 768] ---
                bv_sb = wpool.tile([128, 768], F32, name="bv", tag="bv")
                bvl = b_v.ap()[l]
                nc.sync.dma_start(
                    out=bv_sb,
                    in_=bass.AP(tensor=bvl.tensor, offset=bvl.offset,
                                ap=[[0, 128]] + list(bvl.ap)),
                )
                vloc = [qkvp.tile([128, 768], BF, name=f"vloc{tt}", tag=f"vloc{tt}")
                        for tt in range(2)]
                wv_all = wpool.tile([128, 2, KT, 384], BF, name="wvt", tag="wvt")
                nc.sync.dma_start(out=wv_all, in_=wv[l].rearrange("o p j q -> p o j q"))
                for oh in range(2):
                    for tt in range(2):
                        ps = ps_mm.tile([128, 384], F32, name="mmv", tag="mm")
                        for j in range(KT):
                            nc.tensor.matmul(
                                ps, _r(ln1[j][:, tt * 128 : (tt + 1) * 128]),
                                _r(wv_all[:, oh, j, :]),
                                start=(j == 0), stop=(j == KT - 1))
                        nc.vector.tensor_add(
                            vloc[tt][:, oh * 384 : (oh + 1) * 384], ps,
                            bv_sb[:, oh * 384 : (oh + 1) * 384])
                for tt in range(2):
                    nc.sync.dma_start(
                        out=v_in[tt * 128 : (tt + 1) * 128, :], in_=vloc[tt])

                nc.gpsimd.collective_compute(
                    "AllGather", ALU.bypass, replica_groups=kv_groups,
                    ins=[kv_in.ap()], outs=[kv_out.ap()[0 : 4 * 2 * KVSZ]],
                )
                warm_fill(FILL_LAYER)

                # --- load gathered K (feature-major [768, 1024]) and V ---
                kvo = kv_out.ap()
                for r in range(4):
                    base = r * 2 * KVSZ
                    k_r = kvo[base : base + KVSZ].rearrange(
                        "(j p t) -> p j t", p=128, t=NTOK)
                    nc.sync.dma_start(
                        out=k_all[:, :, r * NTOK : (r + 1) * NTOK], in_=k_r)
                    for a in range(2):
                        v_r = kvo[base + KVSZ + a * KVSZ // 2
                                  : base + KVSZ + (a + 1) * KVSZ // 2].rearrange(
                            "(p h d) -> p h d", p=128, h=12)
                        nc.sync.dma_start(out=v_all[r][:, a, :, 0:64], in_=v_r)

                # --- attention per head (kv blocks paired into 512-wide ops) ---
                yT = [yp.tile([128, NTOK], BF, name=f"y{j}", tag=f"y{j}") for j in range(KT)]
                for h in range(H):
                    p0 = 64 * (h % 2)
                    q_ap = qkT[h // 2][p0 : p0 + 64, :]
                    psy = ps_av.tile([65, NTOK], F32, name="av", tag="av")
                    for kp in range(4):
                        # causal mask injected via identity-stationary matmul
                        # (PE adds mask into PSUM; keeps DVE out of the chain)
                        pss = ps_s.tile([128, 2, NTOK], F32, name="s", tag="s")
                        nc.tensor.matmul(
                            pss, _r(ident_sb),
                            _r(mask_sb[:, 2 * kp : 2 * kp + 2, :]),
                            start=True, stop=False)
                        for half in range(2):
                            kt = 2 * kp + half
                            nc.tensor.matmul(
                                pss[:, half, :],
                                _r(k_all[p0 : p0 + 64, h // 2,
                                         kt * 128 : (kt + 1) * 128]),
                                _r(q_ap), start=False, stop=(half == 1))
                        es = att.tile([128, 2, NTOK], BF, name="es", tag="es")
                        nc.scalar.activation(es, pss, AF.Exp, scale=0.125)
                        for half in range(2):
                            kt = 2 * kp + half
                            nc.tensor.matmul(
                                psy, _r(v_all[kp][:, half, h, :]),
                                _r(es[:, half, :]),
                                start=(kt == 0), stop=(kt == 7))
                    rec = stat.tile([1, NTOK], F32, name="rec", tag="rec")
                    nc.vector.reciprocal(rec, psy[64:65, :])
                    rb = stat.tile([64, NTOK], F32, name="rb", tag="rb")
                    nc.gpsimd.partition_broadcast(rb, rec)
                    nc.vector.tensor_mul(yT[h // 2][p0 : p0 + 64, :],
                                         psy[0:64, :], rb)

                # --- proj + residual ---
                x2_tiles = []
                sq2 = []
                wp_all = wpool.tile([128, KT, KT, 128], BF, name="wp", tag="wp")
                nc.sync.dma_start(out=wp_all, in_=wproj[l].rearrange("o p j q -> p o j q"))
                for ot in range(KT):
                    ps = ps_mm.tile([128, NTOK], F32, name="mm", tag="mm")
                    for j in range(KT):
                        nc.tensor.matmul(
                            ps, _r(wp_all[:, ot, j, :]),
                            _r(yT[j]), start=(j == 0), stop=(j == KT - 1))
                    x2 = resid.tile([128, NTOK], F32R, name=f"x{ot}", tag=f"x{ot}")
                    nc.vector.tensor_scalar_add(ps, ps, bias_sb[:, l, 12 + ot : 13 + ot])
                    nc.vector.tensor_add(x2, ps, x_tiles[ot])
                    sqt = lnp.tile([128, NTOK], F32R, name=f"sq{ot}", tag=f"sq{ot}")
                    nc.vector.tensor_mul(sqt, x2, x2)
                    x2_tiles.append(x2)
                    sq2.append(sqt)

                # --- MLP ---
                ln2 = layernorm(x2_tiles, None, None, folded=True, sq_pre=sq2)
                h_sb = []
                for g in range(2):
                    wf_g = wpool.tile([128, 12, KT, 128], BF, name="wf", tag="wf",
                                      bufs=2)
                    nc.sync.dma_start(
                        out=wf_g,
                        in_=wfc[l, g * 12 : (g + 1) * 12].rearrange("o p j q -> p o j q"))
                    for oi in range(12):
                        ot = g * 12 + oi
                        ps = ps_mm.tile([128, NTOK], F32, name="mm", tag="mm")
                        for j in range(KT):
                            nc.tensor.matmul(
                                ps, _r(wf_g[:, oi, j, :]),
                                _r(ln2[j]), start=(j == 0), stop=(j == KT - 1))
                        hs = hp.tile([128, NTOK], BF, name=f"h{ot}", tag=f"h{ot}")
                        nc.scalar.activation(hs, ps, AF.Gelu_apprx_tanh,
                                             bias=bias_sb[:, l, 18 + ot : 19 + ot])
                        h_sb.append(hs)
                x3_tiles = []
                sq3 = []
                for g2 in range(2):
                    w2_g = w2pool.tile([128, 3, 24, 128], BF, name="w2", tag="w2",
                                       bufs=2)
                    nc.sync.dma_start(
                        out=w2_g,
                        in_=wfc2[l, g2 * 3 : (g2 + 1) * 3].rearrange("o p j q -> p o j q"))
                    for oi in range(3):
                        ot = g2 * 3 + oi
                        ps = ps_mm.tile([128, NTOK], F32, name="mm", tag="mm")
                        for j in range(24):
                            nc.tensor.matmul(
                                ps, _r(w2_g[:, oi, j, :]),
                                _r(h_sb[j]), start=(j == 0), stop=(j == 23))
                        x3 = resid.tile([128, NTOK], F32R, name=f"x{ot}", tag=f"x{ot}")
                        nc.vector.tensor_scalar_add(ps, ps, bias_sb[:, l, 42 + ot : 43 + ot])
                        nc.vector.tensor_add(x3, ps, x2_tiles[ot])
                        sqt = lnp.tile([128, NTOK], F32R, name=f"sq{ot}", tag=f"sq{ot}")
                        nc.vector.tensor_mul(sqt, x3, x3)
                        x3_tiles.append(x3)
                        sq3.append(sqt)
                x_tiles = x3_tiles
                sq_next = sq3

            # --- final LN + AllGather of hidden state ---
            lnf = layernorm(x_tiles, scl[:, 0, :], scl[:, 1, :], sq_pre=sq_next)
            xf_ap = xf_in.ap().rearrange("(p t) -> p t", p=D)
            for j in range(KT):
                nc.sync.dma_start(out=xf_ap[j * 128 : (j + 1) * 128, :], in_=lnf[j])
            nc.gpsimd.collective_compute(
                "AllGather", ALU.bypass, replica_groups=[list(range(NC))],
                ins=[xf_in.ap()], outs=[xf_out.ap()],
            )
            warm_fill(FILL_FINAL)
            nc.sync.dma_start(out=warm_sink.ap(), in_=warm_out)

        # --- logits: out[t, vshard] = xf.T @ wteT ---
        with ExitStack() as lg:
            xfp = lg.enter_context(tc.tile_pool(name="xfp", bufs=1))
            wtep = lg.enter_context(tc.tile_pool(name="wtep", bufs=2))
            outp = lg.enter_context(tc.tile_pool(name="outp", bufs=4))
            ps_l = lg.enter_context(tc.tile_pool(name="ps_l", bufs=4, space="PSUM"))
            xfo = xf_out.ap()
            xf_sb = [xfp.tile([128, B * T], BF, name=f"xf{j}", tag=f"xf{j}") for j in range(KT)]
            for r in range(NC):
                x_r = xfo[r * KVSZ : (r + 1) * KVSZ].rearrange("(p t) -> p t", p=D)
                for j in range(KT):
                    nc.sync.dma_start(
                        out=xf_sb[j][:, r * NTOK : (r + 1) * NTOK],
                        in_=x_r[j * 128 : (j + 1) * 128, :])
            for v2 in range((VT + 1) // 2):
                w = 1024 if 2 * v2 + 1 < VT else 512
                wt_sb = [wtep.tile([128, 1024], BF, name=f"wte{j}", tag=f"wte{j}") for j in range(KT)]
                for j in range(KT):
                    nc.sync.dma_start(out=wt_sb[j][:, 0:512], in_=wteT[j, 2 * v2])
                    if w == 1024:
                        nc.sync.dma_start(out=wt_sb[j][:, 512:1024], in_=wteT[j, 2 * v2 + 1])
                for tt in range(TT):
                    ps = ps_l.tile([128, 1024], F32, name="lg", tag="lg")
                    for h0 in range(0, w, 512):
                        for j in range(KT):
                            nc.tensor.matmul(
                                ps[:, h0 : h0 + 512],
                                _r(xf_sb[j][:, tt * 128 : (tt + 1) * 128]),
                                _r(wt_sb[j][:, h0 : h0 + 512]),
                                start=(j == 0), stop=(j == KT - 1))
                    ot = outp.tile([128, 1024], BF, name="ob", tag="ob")
                    nc.scalar.copy(ot[:, 0:w], ps[:, 0:w])
                    nc.sync.dma_start(
                        out=out.ap()[tt * 128 : (tt + 1) * 128,
                                     v2 * 1024 : v2 * 1024 + w],
                        in_=ot[:, 0:w])

    nc.compile()
    return nc


def prep_inputs(idx, wte, wpe, ln1_s, ln1_b, attn_w, attn_b, proj_w, proj_b,
                ln2_s, ln2_b, fc_w, fc_b, fc2_w, fc2_b, lnf_s, lnf_b):
    f = np.float32
    bf = ml_dtypes.bfloat16
    x0 = (wte[idx.reshape(-1)] + np.tile(wpe, (B, 1))).astype(f)  # [2048, 768]
    wte_pad = np.zeros((NC * VSHARD, D), f)
    wte_pad[:V] = wte
    # fold ln1/ln2 scale+bias into the consuming weights/biases:
    #   W'=W*s[:,None], b'=b+ln_b@W  (kernel then applies plain (x-mu)*rsqrt)
    attn_b = attn_b + np.einsum("ld,ldo->lo", ln1_b, attn_w)
    attn_w = attn_w * ln1_s[:, :, None]
    fc_b = fc_b + np.einsum("ld,ldo->lo", ln2_b, fc_w)
    fc_w = fc_w * ln2_s[:, :, None]
    shared = {
        "wqk": np.ascontiguousarray(
            attn_w[:, :, :1536].reshape(L, KT, 128, 12, 128).transpose(0, 3, 2, 1, 4)).astype(bf),
        "wv": np.ascontiguousarray(
            attn_w[:, :, 1536:].reshape(L, KT, 128, 2, 384).transpose(0, 3, 2, 1, 4)).astype(bf),
        "wproj": np.ascontiguousarray(
            proj_w.reshape(L, KT, 128, KT, 128).transpose(0, 3, 2, 1, 4)).astype(bf),
        "wfc": np.ascontiguousarray(
            fc_w.reshape(L, KT, 128, 24, 128).transpose(0, 3, 2, 1, 4)).astype(bf),
        "wfc2": np.ascontiguousarray(
            fc2_w.reshape(L, 24, 128, KT, 128).transpose(0, 3, 2, 1, 4)).astype(bf),
        "b_qkv": np.ascontiguousarray(
            attn_b[:, :1536].reshape(L, 12, 128).transpose(0, 2, 1)).astype(f),
        "b_v": np.ascontiguousarray(attn_b[:, 1536:]).astype(f),
        "b_proj": np.ascontiguousarray(proj_b.reshape(L, KT, 128).transpose(0, 2, 1)).astype(f),
        "b_fc": np.ascontiguousarray(fc_b.reshape(L, 24, 128).transpose(0, 2, 1)).astype(f),
        "b_fc2": np.ascontiguousarray(fc2_b.reshape(L, KT, 128).transpose(0, 2, 1)).astype(f),
        "s_ln1": np.ascontiguousarray(ln1_s.reshape(L, KT, 128).transpose(0, 2, 1)).astype(f),
        "bi_ln1": np.ascontiguousarray(ln1_b.reshape(L, KT, 128).transpose(0, 2, 1)).astype(f),
        "s_ln2": np.ascontiguousarray(ln2_s.reshape(L, KT, 128).transpose(0, 2, 1)).astype(f),
        "bi_ln2": np.ascontiguousarray(ln2_b.reshape(L, KT, 128).transpose(0, 2, 1)).astype(f),
        "s_lnf": np.ascontiguousarray(lnf_s.reshape(KT, 128).T).astype(f),
        "bi_lnf": np.ascontiguousarray(lnf_b.reshape(KT, 128).T).astype(f),
    }
    in_maps = []
    tk = np.arange(T)[:, None]
    for c in range(NC):
        qs = NTOK * (c % 4)
        m = np.where(tk <= qs + np.arange(NTOK)[None, :], 0.0, MASKVAL).astype(f)
        wsh = wte_pad[c * VSHARD : (c + 1) * VSHARD]  # [6656, 768]
        wteT_t = np.ascontiguousarray(
            wsh.T.reshape(KT, 128, VT, 512).transpose(0, 2, 1, 3)).astype(bf)
        im = dict(shared)
        im["onesd"] = np.ones((128, 65), f)
        im["onesb"] = np.ones((128, 512), bf)
        im["ident"] = np.eye(128, dtype=bf)
        im["x0T"] = np.ascontiguousarray(x0[c * NTOK : (c + 1) * NTOK].T)
        im["mask8"] = m.astype(bf)
        im["wteT"] = wteT_t
        in_maps.append(im)
    return in_maps


def kernel(**inputs):
    inputs = {k: np.asarray(v) for k, v in inputs.items()}
    in_maps = prep_inputs(**inputs)
    if "nc" not in _CACHE:
        _CACHE["nc"] = build_nc()
    res = run_bass_kernel_spmd(_CACHE["nc"], in_maps, list(range(NC)))
    shards = [np.asarray(res.results[c]["out"]).astype(np.float32) for c in range(NC)]
    full = np.concatenate(shards, axis=1)[:, :V]
    return np.ascontiguousarray(full.reshape(B, T, V))
